# revision 8
# baseline (speedup 1.0000x reference)
"""TRN2 Bass kernel for GNN message passing (nn_MessagePassing):

    out = segment_sum(x[src] * edge_weight, dst, num_segments=N)

x: [50000, 64] f32, edge_weight: [1250000] f32, edge_index: [2, 1250000] i64.

Distribution strategy (8 NeuronCores, SPMD):
  - Destination nodes are sharded across the 8 cores (core k owns output rows
    [k*6250, (k+1)*6250)), so no all-reduce is needed: each core computes a
    disjoint output slice and the host concatenates them.
  - x is pair-packed to bf16 on the host: HBM table row p holds
    concat(x[2p], x[2p+1]) as [25000, 128] bf16 (256B rows; dma_gather
    requires elem_size_bytes % 256 == 0).  Pair ids fit int16, so there is
    no table split and gather calls run contiguously.

Per-core device pipeline (v2 — big gathers + on-device scatter matrix):
  - Host buckets edges by (core, 64-node dst window, src parity) and pads
    each bucket to a multiple of 128 edges ("chunks").  Groups of
    GROUP_CHUNKS chunks are gathered with a SINGLE dma_gather call
    (GROUP_CHUNKS*128 indices): SWDGE descriptor generation on the Pool
    engine costs ~1us fixed per call, so fewer/larger calls more than halve
    the desc-gen serial time vs 1024-idx calls.  The SWDGE descriptor ring
    (dynamic_dma_scratch_size) is enlarged to hold a full call.
  - The weighted one-hot scatter matrix S_w (S_w[e, j] = w_e*[dst_local==j])
    is built ON DEVICE by the (otherwise idle) vector engine instead of
    being streamed from HBM (which cost ~23 MB of the bottleneck DMA
    bandwidth): per group, S = is_equal(dstl_bcast, iota) then S *= wgt.
    S is stored chunk-minor ([p, j, c] with stride KGM) so every DVE
    operand's innermost dim is stride-1 2-byte => 2x DVE perf mode.
    Per-chunk metadata (gather idx block | dstl row | wgt row) is packed
    into one int16 stream, one HWDGE copy per group.
  - out_window += S_w_chunk^T @ msg_half accumulates in PSUM in bf16; the
    matmul rhs selects the correct 64-column half of each gathered pair per
    bucket (parity).  lhsT is the strided chunk-minor S slice.
  - Finished windows are copied to SBUF on the Activation engine and DMA'd
    to the output.
"""

import sys

if "/opt/trn_rl_repo" not in sys.path:
    sys.path.insert(0, "/opt/trn_rl_repo")

import numpy as np
from ml_dtypes import bfloat16

import concourse.bacc as bacc
import concourse.mybir as mybir
import concourse.tile as tile
from concourse.bass_utils import run_bass_kernel_spmd

N_CORES = 8
F = 64
W = 64             # dst-window width (one-hot matmul M dim)
CHUNK = 128        # edges per matmul (K dim)
GROUP_CHUNKS = 24  # chunks per group == idxs per dma_gather call / 128
NPAIR = 25000      # x row-pairs


def _cdiv(a, b):
    return (a + b - 1) // b


def _host_prep(x, edge_weight, edge_index):
    N = x.shape[0]
    npc = N // N_CORES
    nw = _cdiv(npc, W)
    src = np.asarray(edge_index[0]).astype(np.int64)
    dst = np.asarray(edge_index[1]).astype(np.int64)
    wgt = np.asarray(edge_weight).astype(np.float32)
    E = src.shape[0]

    core = dst // npc
    rel = dst - core * npc
    win = rel // W
    dstl = rel % W
    par = src & 1
    key = (core * nw + win) * 2 + par
    # sort by src within each bucket: each gather call's accesses become a
    # few ascending sweeps over the x table (HBM row locality) instead of
    # fully random
    order = np.lexsort((src, key))
    key_s = key[order]
    src_s = src[order]
    dstl_s = dstl[order]
    wgt_s = wgt[order]

    nbuckets = N_CORES * nw * 2
    counts = np.bincount(key_s, minlength=nbuckets).reshape(N_CORES, nw, 2)
    nchunks = _cdiv(counts, CHUNK).max(axis=0)      # [nw, 2], SPMD-uniform
    nchunks[:, 0] = np.maximum(nchunks[:, 0], 1)

    # global chunk-major column layout: window-major, parity 0 then 1
    col0 = np.zeros((nw, 2), np.int64)
    c = 0
    for w in range(nw):
        for p in (0, 1):
            col0[w, p] = c
            c += nchunks[w, p]
    K_PAD = int(c)

    gmeta = []
    c0 = 0
    while c0 < K_PAD:
        kg = min(GROUP_CHUNKS, K_PAD - c0)
        gmeta.append((c0, kg))
        c0 += kg

    # per-group matmul work: (window, global chunk, parity, first, last)
    group_matmuls = [[] for _ in gmeta]
    gstarts = [g[0] for g in gmeta]
    for w in range(nw):
        cols = []
        for p in (0, 1):
            for j in range(int(nchunks[w, p])):
                cols.append((int(col0[w, p]) + j, p))
        for j, (cg, p) in enumerate(cols):
            g = np.searchsorted(gstarts, cg, "right") - 1
            group_matmuls[g].append(
                (w, cg, p, j == 0, j == len(cols) - 1))

    bstart = np.concatenate([[0], np.cumsum(counts.reshape(-1))])
    slot_base = np.zeros(nbuckets, np.int64)
    for ci in range(N_CORES):
        for w in range(nw):
            for p in (0, 1):
                slot_base[(ci * nw + w) * 2 + p] = col0[w, p] * CHUNK
    rank = np.arange(E) - bstart[key_s]
    slot = slot_base[key_s] + rank

    x32 = np.asarray(x, dtype=np.float32)
    xp = np.ascontiguousarray(x32.astype(bfloat16).reshape(NPAIR, 2 * F))

    # constant chunk-minor iota: iota[p, j*KGM + c] = j  (bf16, as int16 bits)
    KGM = GROUP_CHUNKS
    iota_row = (np.arange(W * KGM) // KGM).astype(bfloat16).view(np.int16)
    iota16 = np.ascontiguousarray(np.tile(iota_row, (128, 1)))

    in_maps = []
    for ci in range(N_CORES):
        lo = np.searchsorted(key_s, ci * nw * 2, "left")
        hi_ = np.searchsorted(key_s, (ci + 1) * nw * 2, "left")
        sl = slot[lo:hi_]
        idx_stream = np.zeros(K_PAD * CHUNK, np.int32)
        idx_stream[sl] = src_s[lo:hi_] >> 1
        idx16 = np.tile(
            idx_stream.astype(np.int16).reshape(K_PAD * 8, 16).T, (8, 1))

        # per-chunk dst-local and weight planes, [128, K_PAD] bf16-as-int16
        e_in_chunk = (sl % CHUNK).astype(np.int64)
        chunk_id = sl // CHUNK
        dstl16 = np.zeros((128, K_PAD), np.int16)
        wgt16 = np.zeros((128, K_PAD), np.int16)
        dstl16[e_in_chunk, chunk_id] = (
            dstl_s[lo:hi_].astype(bfloat16).view(np.int16))
        wgt16[e_in_chunk, chunk_id] = (
            wgt_s[lo:hi_].astype(bfloat16).view(np.int16))

        # pack per group: [idx kg*8 | dstl kg | wgt kg] -> [128, K_PAD*10]
        meta = np.empty((128, K_PAD * 10), np.int16)
        off = 0
        for (c0g, kg) in gmeta:
            meta[:, off:off + kg * 8] = idx16[:, c0g * 8:(c0g + kg) * 8]
            meta[:, off + kg * 8:off + kg * 9] = dstl16[:, c0g:c0g + kg]
            meta[:, off + kg * 9:off + kg * 10] = wgt16[:, c0g:c0g + kg]
            off += kg * 10

        in_maps.append({
            "xp": xp,
            "meta": np.ascontiguousarray(meta),
            "iota": iota16,
        })

    meta_info = dict(N=N, npc=npc, nw=nw, K_PAD=K_PAD,
                     nchunks=nchunks, col0=col0, gmeta=gmeta,
                     group_matmuls=group_matmuls)
    return in_maps, meta_info


def _build_program(meta, reps=1, msg_bufs=12, s_bufs=6, meta_bufs=8,
                   gather_chunks=8, scratch=16384):
    npc, nw, K_PAD = meta["npc"], meta["nw"], meta["K_PAD"]
    gmeta = meta["gmeta"]
    group_matmuls = meta["group_matmuls"]
    f32, bf16, i16 = mybir.dt.float32, mybir.dt.bfloat16, mybir.dt.int16
    KGM = GROUP_CHUNKS

    nc = bacc.Bacc("TRN2", target_bir_lowering=False, debug=False,
                   num_devices=N_CORES, num_swdge_queues=4,
                   dynamic_dma_scratch_size=scratch)
    xp_d = nc.dram_tensor("xp", [NPAIR, 2 * F], bf16, kind="ExternalInput")
    meta_d = nc.dram_tensor("meta", [128, K_PAD * 10], i16,
                            kind="ExternalInput")
    iota_d = nc.dram_tensor("iota", [128, W * KGM], i16,
                            kind="ExternalInput")
    out_d = nc.dram_tensor("out", [npc, F], f32, kind="ExternalOutput")

    with tile.TileContext(nc) as tc:
        with (
            tc.tile_pool(name="big", bufs=msg_bufs) as big,
            tc.tile_pool(name="cpool", bufs=1) as cpool,
            tc.tile_pool(name="opool", bufs=2) as opool,
            tc.tile_pool(name="psum", bufs=8, space="PSUM") as pp,
        ):
            qctr = [0]
            iota_t = cpool.tile([128, W * KGM], i16, tag="iota")
            nc.sync.dma_start(out=iota_t[:], in_=iota_d.ap()[:, :])
            iota_b = iota_t[:].bitcast(bf16).rearrange(
                "p (j c) -> p j c", c=KGM)

            def body():
                o_t = opool.tile([W, nw * F], f32, tag="obig")
                ps_open = {}
                moff = [0]
                for g, (c0, kg) in enumerate(gmeta):
                    mo = moff[0]
                    moff[0] += kg * 10
                    meta_t = big.tile([128, kg * 10], i16, tag="meta",
                                      bufs=meta_bufs)
                    nc.sync.dma_start(
                        out=meta_t[:], in_=meta_d.ap()[:, mo:mo + kg * 10])
                    msg_t = big.tile([128, kg * 2 * F], bf16, tag="msg")
                    done = 0
                    while done < kg:
                        blk = min(gather_chunks, kg - done)
                        nc.gpsimd.dma_gather(
                            out_ap=msg_t[:, done * 2 * F:
                                         (done + blk) * 2 * F].rearrange(
                                "p (c f) -> p c f", f=2 * F),
                            in_ap=xp_d.ap()[:],
                            idxs_ap=meta_t[:, done * 8:(done + blk) * 8],
                            num_idxs=blk * CHUNK,
                            num_idxs_reg=blk * CHUNK,
                            elem_size=2 * F,
                            queue_num=qctr[0] % 4,
                        )
                        qctr[0] += 1
                        done += blk
                    # on-device weighted one-hot, chunk-minor [p, j, c]
                    S_t = big.tile([128, W * KGM], bf16, tag="S",
                                   bufs=s_bufs)
                    S_3 = S_t[:].rearrange("p (j c) -> p j c", c=KGM)
                    dstl_b = meta_t[:, kg * 8:kg * 9].bitcast(bf16)
                    wgt_b = meta_t[:, kg * 9:kg * 10].bitcast(bf16)
                    nc.vector.tensor_tensor(
                        out=S_3[:, :, :kg],
                        in0=dstl_b.rearrange("p (one c) -> p one c",
                                             one=1).broadcast_to(
                            [128, W, kg]),
                        in1=iota_b[:, :, :kg],
                        op=mybir.AluOpType.is_equal)
                    nc.vector.tensor_tensor(
                        out=S_3[:, :, :kg],
                        in0=S_3[:, :, :kg],
                        in1=wgt_b.rearrange("p (one c) -> p one c",
                                            one=1).broadcast_to(
                            [128, W, kg]),
                        op=mybir.AluOpType.mult)
                    for (w, cg, p, first, last) in group_matmuls[g]:
                        if first:
                            ps_open[w] = pp.tile([W, F], f32, tag="ps",
                                                 name="ps")
                        cc = cg - c0
                        nc.tensor.matmul(
                            out=ps_open[w][:],
                            lhsT=S_3[:, :, cc],
                            rhs=msg_t[:, cc * 2 * F + p * F:
                                      cc * 2 * F + p * F + F],
                            start=first, stop=last)
                        if last:
                            nc.scalar.activation(
                                out=o_t[:, w * F:(w + 1) * F],
                                in_=ps_open.pop(w)[:],
                                func=mybir.ActivationFunctionType.Copy)
                            rows = min(W, npc - w * W)
                            nc.sync.dma_start(
                                out=out_d.ap()[w * W:w * W + rows, :],
                                in_=o_t[:rows, w * F:(w + 1) * F])

            for _ in range(reps):
                body()
    nc.compile()
    return nc


def build_for_inputs(x, edge_weight, edge_index, reps=1, **knobs):
    """Exposed for test harnesses: returns (nc, in_maps, meta)."""
    in_maps, meta = _host_prep(x, edge_weight, edge_index)
    nc = _build_program(meta, reps=reps, **knobs)
    return nc, in_maps, meta


def kernel(x, edge_weight, edge_index):
    x = np.asarray(x)
    nc, in_maps, _meta = build_for_inputs(x, edge_weight, edge_index)
    res = run_bass_kernel_spmd(nc, in_maps, core_ids=list(range(N_CORES)))
    out = np.concatenate(
        [res.results[c]["out"] for c in range(N_CORES)], axis=0)
    return out.astype(np.float32)


# revision 15
# speedup vs baseline: 1.1597x; 1.1597x over previous
"""TRN2 Bass kernel for GNN message passing (nn_MessagePassing):

    out = segment_sum(x[src] * edge_weight, dst, num_segments=N)

x: [50000, 64] f32, edge_weight: [1250000] f32, edge_index: [2, 1250000] i64.

Distribution strategy (8 NeuronCores, SPMD):
  - Destination nodes are sharded across the 8 cores (core k owns output rows
    [k*6250, (k+1)*6250)), so no all-reduce is needed: each core computes a
    disjoint output slice and the host concatenates them.
  - x is pair-packed to bf16 on the host: HBM table row p holds
    concat(x[2p], x[2p+1]) as [25000, 128] bf16 (256B rows; dma_gather
    requires elem_size_bytes % 256 == 0).  Pair ids fit int16, so there is
    no table split and gather calls run contiguously.

Per-core device pipeline (v2 — big gathers + on-device scatter matrix):
  - Host buckets edges by (core, 64-node dst window, src parity) and pads
    each bucket to a multiple of 128 edges ("chunks").  Groups of
    GROUP_CHUNKS chunks are gathered with a SINGLE dma_gather call
    (GROUP_CHUNKS*128 indices): SWDGE descriptor generation on the Pool
    engine costs ~1us fixed per call, so fewer/larger calls more than halve
    the desc-gen serial time vs 1024-idx calls.  The SWDGE descriptor ring
    (dynamic_dma_scratch_size) is enlarged to hold a full call.
  - The weighted one-hot scatter matrix S_w (S_w[e, j] = w_e*[dst_local==j])
    is built ON DEVICE by the (otherwise idle) vector engine instead of
    being streamed from HBM (which cost ~23 MB of the bottleneck DMA
    bandwidth): per group, S = is_equal(dstl_bcast, iota) then S *= wgt.
    S is stored chunk-minor ([p, j, c] with stride KGM) so every DVE
    operand's innermost dim is stride-1 2-byte => 2x DVE perf mode.
    Per-chunk metadata (gather idx block | dstl row | wgt row) is packed
    into one int16 stream, one HWDGE copy per group.
  - out_window += S_w_chunk^T @ msg_half accumulates in PSUM in bf16; the
    matmul rhs selects the correct 64-column half of each gathered pair per
    bucket (parity).  lhsT is the strided chunk-minor S slice.
  - Finished windows are copied to SBUF on the Activation engine and DMA'd
    to the output.
"""

import sys

if "/opt/trn_rl_repo" not in sys.path:
    sys.path.insert(0, "/opt/trn_rl_repo")

import numpy as np
from ml_dtypes import bfloat16

import concourse.bacc as bacc
import concourse.mybir as mybir
import concourse.tile as tile
from concourse.bass_utils import run_bass_kernel_spmd

N_CORES = 8
F = 64
W = 64             # dst-window width (one-hot matmul M dim)
CHUNK = 128        # edges per matmul (K dim)
GROUP_CHUNKS = 24  # chunks per group == idxs per dma_gather call / 128
NPAIR = 25000      # x row-pairs


def _cdiv(a, b):
    return (a + b - 1) // b


def _host_prep(x, edge_weight, edge_index, seq_idx=False):
    N = x.shape[0]
    npc = N // N_CORES
    nw = _cdiv(npc, W)
    src = np.asarray(edge_index[0]).astype(np.int64)
    dst = np.asarray(edge_index[1]).astype(np.int64)
    wgt = np.asarray(edge_weight).astype(np.float32)
    E = src.shape[0]

    core = dst // npc
    rel = dst - core * npc
    win = rel // W
    dstl = rel % W
    par = src & 1
    key = (core * nw + win) * 2 + par
    # sort by src within each bucket: each gather call's accesses become a
    # few ascending sweeps over the x table (HBM row locality) instead of
    # fully random
    order = np.lexsort((src, key))
    key_s = key[order]
    src_s = src[order]
    dstl_s = dstl[order]
    wgt_s = wgt[order]

    nbuckets = N_CORES * nw * 2
    counts = np.bincount(key_s, minlength=nbuckets).reshape(N_CORES, nw, 2)
    nchunks = _cdiv(counts, CHUNK).max(axis=0)      # [nw, 2], SPMD-uniform
    nchunks[:, 0] = np.maximum(nchunks[:, 0], 1)

    # global chunk-major column layout: window-major, parity 0 then 1
    col0 = np.zeros((nw, 2), np.int64)
    c = 0
    for w in range(nw):
        for p in (0, 1):
            col0[w, p] = c
            c += nchunks[w, p]
    K_PAD = int(c)

    gmeta = []
    c0 = 0
    while c0 < K_PAD:
        kg = min(GROUP_CHUNKS, K_PAD - c0)
        gmeta.append((c0, kg))
        c0 += kg

    # per-group matmul work: (window, global chunk, parity, first, last)
    group_matmuls = [[] for _ in gmeta]
    gstarts = [g[0] for g in gmeta]
    for w in range(nw):
        cols = []
        for p in (0, 1):
            for j in range(int(nchunks[w, p])):
                cols.append((int(col0[w, p]) + j, p))
        for j, (cg, p) in enumerate(cols):
            g = np.searchsorted(gstarts, cg, "right") - 1
            group_matmuls[g].append(
                (w, cg, p, j == 0, j == len(cols) - 1))

    bstart = np.concatenate([[0], np.cumsum(counts.reshape(-1))])
    slot_base = np.zeros(nbuckets, np.int64)
    for ci in range(N_CORES):
        for w in range(nw):
            for p in (0, 1):
                slot_base[(ci * nw + w) * 2 + p] = col0[w, p] * CHUNK
    rank = np.arange(E) - bstart[key_s]
    slot = slot_base[key_s] + rank

    x32 = np.asarray(x, dtype=np.float32)
    xp = np.ascontiguousarray(x32.astype(bfloat16).reshape(NPAIR, 2 * F))

    # constant chunk-minor iota: iota[p, j*KGM + c] = j  (bf16, as int16 bits)
    KGM = GROUP_CHUNKS
    iota_row = (np.arange(W * KGM) // KGM).astype(bfloat16).view(np.int16)
    iota16 = np.ascontiguousarray(np.tile(iota_row, (128, 1)))

    in_maps = []
    for ci in range(N_CORES):
        lo = np.searchsorted(key_s, ci * nw * 2, "left")
        hi_ = np.searchsorted(key_s, (ci + 1) * nw * 2, "left")
        sl = slot[lo:hi_]
        idx_stream = np.zeros(K_PAD * CHUNK, np.int32)
        idx_stream[sl] = src_s[lo:hi_] >> 1
        if seq_idx:  # timing probe: same descs, sequential table reads
            idx_stream = np.arange(K_PAD * CHUNK, dtype=np.int32) % NPAIR
        idx16 = np.tile(
            idx_stream.astype(np.int16).reshape(K_PAD * 8, 16).T, (8, 1))

        # per-chunk dst-local and weight planes, [128, K_PAD] bf16-as-int16
        e_in_chunk = (sl % CHUNK).astype(np.int64)
        chunk_id = sl // CHUNK
        dstl16 = np.zeros((128, K_PAD), np.int16)
        wgt16 = np.zeros((128, K_PAD), np.int16)
        dstl16[e_in_chunk, chunk_id] = (
            dstl_s[lo:hi_].astype(bfloat16).view(np.int16))
        wgt16[e_in_chunk, chunk_id] = (
            wgt_s[lo:hi_].astype(bfloat16).view(np.int16))

        # pack per group: [idx kg*8 | dstl kg | wgt kg] -> [128, K_PAD*10]
        meta = np.empty((128, K_PAD * 10), np.int16)
        off = 0
        for (c0g, kg) in gmeta:
            meta[:, off:off + kg * 8] = idx16[:, c0g * 8:(c0g + kg) * 8]
            meta[:, off + kg * 8:off + kg * 9] = dstl16[:, c0g:c0g + kg]
            meta[:, off + kg * 9:off + kg * 10] = wgt16[:, c0g:c0g + kg]
            off += kg * 10

        in_maps.append({
            "xp": xp,
            "meta": np.ascontiguousarray(meta),
            "iota": iota16,
        })

    meta_info = dict(N=N, npc=npc, nw=nw, K_PAD=K_PAD,
                     nchunks=nchunks, col0=col0, gmeta=gmeta,
                     group_matmuls=group_matmuls)
    return in_maps, meta_info


def _build_program(meta, reps=1, msg_bufs=12, s_bufs=6, meta_bufs=8,
                   gather_chunks=8, scratch=16384, skip_gather=False,
                   skip_matmul=False):
    npc, nw, K_PAD = meta["npc"], meta["nw"], meta["K_PAD"]
    gmeta = meta["gmeta"]
    group_matmuls = meta["group_matmuls"]
    f32, bf16, i16 = mybir.dt.float32, mybir.dt.bfloat16, mybir.dt.int16
    KGM = GROUP_CHUNKS

    nc = bacc.Bacc("TRN2", target_bir_lowering=False, debug=False,
                   num_devices=N_CORES, num_swdge_queues=4,
                   dynamic_dma_scratch_size=scratch)
    xp_d = nc.dram_tensor("xp", [NPAIR, 2 * F], bf16, kind="ExternalInput")
    meta_d = nc.dram_tensor("meta", [128, K_PAD * 10], i16,
                            kind="ExternalInput")
    iota_d = nc.dram_tensor("iota", [128, W * KGM], i16,
                            kind="ExternalInput")
    out_d = nc.dram_tensor("out", [npc, F], f32, kind="ExternalOutput")

    with tile.TileContext(nc) as tc:
        with (
            tc.tile_pool(name="big", bufs=msg_bufs) as big,
            tc.tile_pool(name="cpool", bufs=1) as cpool,
            tc.tile_pool(name="opool", bufs=2) as opool,
            tc.tile_pool(name="psum", bufs=8, space="PSUM") as pp,
        ):
            qctr = [0]
            iota_t = cpool.tile([128, W * KGM], i16, tag="iota")
            nc.sync.dma_start(out=iota_t[:], in_=iota_d.ap()[:, :])
            iota_b = iota_t[:].bitcast(bf16).rearrange(
                "p (j c) -> p j c", c=KGM)

            def body():
                o_t = opool.tile([W, nw * F], f32, tag="obig")
                ps_open = {}
                moff = [0]
                for g, (c0, kg) in enumerate(gmeta):
                    mo = moff[0]
                    moff[0] += kg * 10
                    meta_t = big.tile([128, kg * 10], i16, tag="meta",
                                      bufs=meta_bufs)
                    nc.sync.dma_start(
                        out=meta_t[:], in_=meta_d.ap()[:, mo:mo + kg * 10])
                    msg_t = big.tile([128, kg * 2 * F], bf16, tag="msg")
                    done = 0
                    while done < kg and not skip_gather:
                        blk = min(gather_chunks, kg - done)
                        nc.gpsimd.dma_gather(
                            out_ap=msg_t[:, done * 2 * F:
                                         (done + blk) * 2 * F].rearrange(
                                "p (c f) -> p c f", f=2 * F),
                            in_ap=xp_d.ap()[:],
                            idxs_ap=meta_t[:, done * 8:(done + blk) * 8],
                            num_idxs=blk * CHUNK,
                            num_idxs_reg=blk * CHUNK,
                            elem_size=2 * F,
                            queue_num=qctr[0] % 4,
                        )
                        qctr[0] += 1
                        done += blk
                    # on-device weighted one-hot, chunk-minor [p, j, c]
                    S_t = big.tile([128, W * KGM], bf16, tag="S",
                                   bufs=s_bufs)
                    S_3 = S_t[:].rearrange("p (j c) -> p j c", c=KGM)
                    dstl_b = meta_t[:, kg * 8:kg * 9].bitcast(bf16)
                    wgt_b = meta_t[:, kg * 9:kg * 10].bitcast(bf16)
                    nc.vector.tensor_tensor(
                        out=S_3[:, :, :kg],
                        in0=dstl_b.rearrange("p (one c) -> p one c",
                                             one=1).broadcast_to(
                            [128, W, kg]),
                        in1=iota_b[:, :, :kg],
                        op=mybir.AluOpType.is_equal)
                    nc.vector.tensor_tensor(
                        out=S_3[:, :, :kg],
                        in0=S_3[:, :, :kg],
                        in1=wgt_b.rearrange("p (one c) -> p one c",
                                            one=1).broadcast_to(
                            [128, W, kg]),
                        op=mybir.AluOpType.mult)
                    for (w, cg, p, first, last) in group_matmuls[g]:
                        if skip_matmul:
                            break
                        if first:
                            ps_open[w] = pp.tile([W, F], f32, tag="ps",
                                                 name="ps")
                        cc = cg - c0
                        nc.tensor.matmul(
                            out=ps_open[w][:],
                            lhsT=S_3[:, :, cc],
                            rhs=msg_t[:, cc * 2 * F + p * F:
                                      cc * 2 * F + p * F + F],
                            start=first, stop=last)
                        if last:
                            nc.scalar.activation(
                                out=o_t[:, w * F:(w + 1) * F],
                                in_=ps_open.pop(w)[:],
                                func=mybir.ActivationFunctionType.Copy)
                # batched output write: one strided DMA for the full windows
                # + a tail DMA for the partial last window (saves ~96 HWDGE
                # fixed costs + semaphore round-trips vs per-window DMAs)
                nwf = npc // W                  # full windows
                rows_t = npc - nwf * W          # rows in partial last window
                nc.sync.dma_start(
                    out=out_d.ap()[:nwf * W, :].rearrange(
                        "(w r) f -> r w f", r=W),
                    in_=o_t[:, :nwf * F].rearrange(
                        "r (w f) -> r w f", f=F))
                if rows_t:
                    nc.sync.dma_start(
                        out=out_d.ap()[nwf * W:npc, :],
                        in_=o_t[:rows_t, nwf * F:(nwf + 1) * F])

            for _ in range(reps):
                body()
    nc.compile()
    return nc


def build_for_inputs(x, edge_weight, edge_index, reps=1, seq_idx=False,
                     **knobs):
    """Exposed for test harnesses: returns (nc, in_maps, meta)."""
    in_maps, meta = _host_prep(x, edge_weight, edge_index, seq_idx=seq_idx)
    nc = _build_program(meta, reps=reps, **knobs)
    return nc, in_maps, meta


def kernel(x, edge_weight, edge_index):
    x = np.asarray(x)
    nc, in_maps, _meta = build_for_inputs(x, edge_weight, edge_index)
    res = run_bass_kernel_spmd(nc, in_maps, core_ids=list(range(N_CORES)))
    out = np.concatenate(
        [res.results[c]["out"] for c in range(N_CORES)], axis=0)
    return out.astype(np.float32)


# revision 16
# speedup vs baseline: 1.8442x; 1.5903x over previous
"""TRN2 Bass kernel for GNN message passing (nn_MessagePassing):

    out = segment_sum(x[src] * edge_weight, dst, num_segments=N)

x: [50000, 64] f32, edge_weight: [1250000] f32, edge_index: [2, 1250000] i64.

Distribution strategy (8 NeuronCores, SPMD):
  - Destination nodes are sharded across the 8 cores (core k owns output rows
    [k*6250, (k+1)*6250)), so no all-reduce is needed: each core computes a
    disjoint output slice and the host concatenates them.
  - x is pair-packed to bf16 on the host: HBM table row p holds
    concat(x[2p], x[2p+1]) as [25000, 128] bf16 (256B rows; dma_gather
    requires elem_size_bytes % 256 == 0).  Pair ids fit int16, so there is
    no table split and gather calls run contiguously.

Per-core device pipeline (v2 — big gathers + on-device scatter matrix):
  - Host buckets edges by (core, 64-node dst window, src parity) and pads
    each bucket to a multiple of 128 edges ("chunks").  Groups of
    GROUP_CHUNKS chunks are gathered with a SINGLE dma_gather call
    (GROUP_CHUNKS*128 indices): SWDGE descriptor generation on the Pool
    engine costs ~1us fixed per call, so fewer/larger calls more than halve
    the desc-gen serial time vs 1024-idx calls.  The SWDGE descriptor ring
    (dynamic_dma_scratch_size) is enlarged to hold a full call.
  - The weighted one-hot scatter matrix S_w (S_w[e, j] = w_e*[dst_local==j])
    is built ON DEVICE by the (otherwise idle) vector engine instead of
    being streamed from HBM (which cost ~23 MB of the bottleneck DMA
    bandwidth): per group, S = is_equal(dstl_bcast, iota) then S *= wgt.
    S is stored chunk-minor ([p, j, c] with stride KGM) so every DVE
    operand's innermost dim is stride-1 2-byte => 2x DVE perf mode.
    Per-chunk metadata (gather idx block | dstl row | wgt row) is packed
    into one int16 stream, one HWDGE copy per group.
  - out_window += S_w_chunk^T @ msg_half accumulates in PSUM in bf16; the
    matmul rhs selects the correct 64-column half of each gathered pair per
    bucket (parity).  lhsT is the strided chunk-minor S slice.
  - Finished windows are copied to SBUF on the Activation engine and DMA'd
    to the output.
"""

import sys

if "/opt/trn_rl_repo" not in sys.path:
    sys.path.insert(0, "/opt/trn_rl_repo")

import numpy as np
from ml_dtypes import bfloat16

import concourse.bacc as bacc
import concourse.mybir as mybir
import concourse.tile as tile
from concourse.bass_utils import run_bass_kernel_spmd

N_CORES = 8
F = 64
W = 64             # dst-window width (one-hot matmul M dim)
CHUNK = 128        # edges per matmul (K dim)
GROUP_CHUNKS = 48  # chunks per group == idxs per dma_gather call / 128
NPAIR = 25000      # x row-pairs


def _cdiv(a, b):
    return (a + b - 1) // b


def _host_prep(x, edge_weight, edge_index, seq_idx=False):
    N = x.shape[0]
    npc = N // N_CORES
    nw = _cdiv(npc, W)
    src = np.asarray(edge_index[0]).astype(np.int64)
    dst = np.asarray(edge_index[1]).astype(np.int64)
    wgt = np.asarray(edge_weight).astype(np.float32)
    E = src.shape[0]

    core = dst // npc
    rel = dst - core * npc
    win = rel // W
    dstl = rel % W
    par = src & 1
    key = (core * nw + win) * 2 + par
    # sort by src within each bucket: each gather call's accesses become a
    # few ascending sweeps over the x table (HBM row locality) instead of
    # fully random
    order = np.lexsort((src, key))
    key_s = key[order]
    src_s = src[order]
    dstl_s = dstl[order]
    wgt_s = wgt[order]

    nbuckets = N_CORES * nw * 2
    counts = np.bincount(key_s, minlength=nbuckets).reshape(N_CORES, nw, 2)
    nchunks = _cdiv(counts, CHUNK).max(axis=0)      # [nw, 2], SPMD-uniform
    nchunks[:, 0] = np.maximum(nchunks[:, 0], 1)

    # global chunk-major column layout: window-major, parity 0 then 1
    col0 = np.zeros((nw, 2), np.int64)
    c = 0
    for w in range(nw):
        for p in (0, 1):
            col0[w, p] = c
            c += nchunks[w, p]
    K_PAD = int(c)

    gmeta = []
    c0 = 0
    while c0 < K_PAD:
        kg = min(GROUP_CHUNKS, K_PAD - c0)
        gmeta.append((c0, kg))
        c0 += kg

    # per-group matmul work: (window, global chunk, parity, first, last)
    group_matmuls = [[] for _ in gmeta]
    gstarts = [g[0] for g in gmeta]
    for w in range(nw):
        cols = []
        for p in (0, 1):
            for j in range(int(nchunks[w, p])):
                cols.append((int(col0[w, p]) + j, p))
        for j, (cg, p) in enumerate(cols):
            g = np.searchsorted(gstarts, cg, "right") - 1
            group_matmuls[g].append(
                (w, cg, p, j == 0, j == len(cols) - 1))

    bstart = np.concatenate([[0], np.cumsum(counts.reshape(-1))])
    slot_base = np.zeros(nbuckets, np.int64)
    for ci in range(N_CORES):
        for w in range(nw):
            for p in (0, 1):
                slot_base[(ci * nw + w) * 2 + p] = col0[w, p] * CHUNK
    rank = np.arange(E) - bstart[key_s]
    slot = slot_base[key_s] + rank

    x32 = np.asarray(x, dtype=np.float32)
    xp = np.ascontiguousarray(x32.astype(bfloat16).reshape(NPAIR, 2 * F))

    # constant chunk-minor iota: iota[p, j*KGM + c] = j  (bf16, as int16 bits)
    KGM = GROUP_CHUNKS
    iota_row = (np.arange(W * KGM) // KGM).astype(bfloat16).view(np.int16)
    iota16 = np.ascontiguousarray(np.tile(iota_row, (128, 1)))

    in_maps = []
    for ci in range(N_CORES):
        lo = np.searchsorted(key_s, ci * nw * 2, "left")
        hi_ = np.searchsorted(key_s, (ci + 1) * nw * 2, "left")
        sl = slot[lo:hi_]
        idx_stream = np.zeros(K_PAD * CHUNK, np.int32)
        idx_stream[sl] = src_s[lo:hi_] >> 1
        if seq_idx:  # timing probe: same descs, sequential table reads
            idx_stream = np.arange(K_PAD * CHUNK, dtype=np.int32) % NPAIR
        idx16 = np.tile(
            idx_stream.astype(np.int16).reshape(K_PAD * 8, 16).T, (8, 1))

        # per-chunk dst-local and weight planes, [128, K_PAD] bf16-as-int16
        e_in_chunk = (sl % CHUNK).astype(np.int64)
        chunk_id = sl // CHUNK
        dstl16 = np.zeros((128, K_PAD), np.int16)
        wgt16 = np.zeros((128, K_PAD), np.int16)
        dstl16[e_in_chunk, chunk_id] = (
            dstl_s[lo:hi_].astype(bfloat16).view(np.int16))
        wgt16[e_in_chunk, chunk_id] = (
            wgt_s[lo:hi_].astype(bfloat16).view(np.int16))

        # pack per group: [idx kg*8 | dstl kg | wgt kg] -> [128, K_PAD*10]
        meta = np.empty((128, K_PAD * 10), np.int16)
        off = 0
        for (c0g, kg) in gmeta:
            meta[:, off:off + kg * 8] = idx16[:, c0g * 8:(c0g + kg) * 8]
            meta[:, off + kg * 8:off + kg * 9] = dstl16[:, c0g:c0g + kg]
            meta[:, off + kg * 9:off + kg * 10] = wgt16[:, c0g:c0g + kg]
            off += kg * 10

        in_maps.append({
            "xp": xp,
            "meta": np.ascontiguousarray(meta),
            "iota": iota16,
        })

    meta_info = dict(N=N, npc=npc, nw=nw, K_PAD=K_PAD,
                     nchunks=nchunks, col0=col0, gmeta=gmeta,
                     group_matmuls=group_matmuls)
    return in_maps, meta_info


def _build_program(meta, reps=1, msg_bufs=6, s_bufs=4, meta_bufs=4,
                   gather_chunks=8, scratch=16384, skip_gather=False,
                   skip_matmul=False):
    npc, nw, K_PAD = meta["npc"], meta["nw"], meta["K_PAD"]
    gmeta = meta["gmeta"]
    group_matmuls = meta["group_matmuls"]
    f32, bf16, i16 = mybir.dt.float32, mybir.dt.bfloat16, mybir.dt.int16
    KGM = GROUP_CHUNKS

    nc = bacc.Bacc("TRN2", target_bir_lowering=False, debug=False,
                   num_devices=N_CORES, num_swdge_queues=4,
                   dynamic_dma_scratch_size=scratch)
    xp_d = nc.dram_tensor("xp", [NPAIR, 2 * F], bf16, kind="ExternalInput")
    meta_d = nc.dram_tensor("meta", [128, K_PAD * 10], i16,
                            kind="ExternalInput")
    iota_d = nc.dram_tensor("iota", [128, W * KGM], i16,
                            kind="ExternalInput")
    out_d = nc.dram_tensor("out", [npc, F], f32, kind="ExternalOutput")

    with tile.TileContext(nc) as tc:
        with (
            tc.tile_pool(name="big", bufs=msg_bufs) as big,
            tc.tile_pool(name="cpool", bufs=1) as cpool,
            tc.tile_pool(name="opool", bufs=2) as opool,
            tc.tile_pool(name="psum", bufs=8, space="PSUM") as pp,
        ):
            qctr = [0]
            iota_t = cpool.tile([128, W * KGM], i16, tag="iota")
            nc.sync.dma_start(out=iota_t[:], in_=iota_d.ap()[:, :])
            iota_b = iota_t[:].bitcast(bf16).rearrange(
                "p (j c) -> p j c", c=KGM)

            def body():
                o_t = opool.tile([W, nw * F], f32, tag="obig")
                ps_open = {}
                moff = [0]
                for g, (c0, kg) in enumerate(gmeta):
                    mo = moff[0]
                    moff[0] += kg * 10
                    meta_t = big.tile([128, kg * 10], i16, tag="meta",
                                      bufs=meta_bufs)
                    nc.sync.dma_start(
                        out=meta_t[:], in_=meta_d.ap()[:, mo:mo + kg * 10])
                    msg_t = big.tile([128, kg * 2 * F], bf16, tag="msg")
                    done = 0
                    while done < kg and not skip_gather:
                        blk = min(gather_chunks, kg - done)
                        nc.gpsimd.dma_gather(
                            out_ap=msg_t[:, done * 2 * F:
                                         (done + blk) * 2 * F].rearrange(
                                "p (c f) -> p c f", f=2 * F),
                            in_ap=xp_d.ap()[:],
                            idxs_ap=meta_t[:, done * 8:(done + blk) * 8],
                            num_idxs=blk * CHUNK,
                            num_idxs_reg=blk * CHUNK,
                            elem_size=2 * F,
                            queue_num=qctr[0] % 4,
                        )
                        qctr[0] += 1
                        done += blk
                    # on-device weighted one-hot, chunk-minor [p, j, c]
                    S_t = big.tile([128, W * KGM], bf16, tag="S",
                                   bufs=s_bufs)
                    S_3 = S_t[:].rearrange("p (j c) -> p j c", c=KGM)
                    dstl_b = meta_t[:, kg * 8:kg * 9].bitcast(bf16)
                    wgt_b = meta_t[:, kg * 9:kg * 10].bitcast(bf16)
                    nc.vector.tensor_tensor(
                        out=S_3[:, :, :kg],
                        in0=dstl_b.rearrange("p (one c) -> p one c",
                                             one=1).broadcast_to(
                            [128, W, kg]),
                        in1=iota_b[:, :, :kg],
                        op=mybir.AluOpType.is_equal)
                    nc.vector.tensor_tensor(
                        out=S_3[:, :, :kg],
                        in0=S_3[:, :, :kg],
                        in1=wgt_b.rearrange("p (one c) -> p one c",
                                            one=1).broadcast_to(
                            [128, W, kg]),
                        op=mybir.AluOpType.mult)
                    for (w, cg, p, first, last) in group_matmuls[g]:
                        if skip_matmul:
                            break
                        if first:
                            ps_open[w] = pp.tile([W, F], f32, tag="ps",
                                                 name="ps")
                        cc = cg - c0
                        nc.tensor.matmul(
                            out=ps_open[w][:],
                            lhsT=S_3[:, :, cc],
                            rhs=msg_t[:, cc * 2 * F + p * F:
                                      cc * 2 * F + p * F + F],
                            start=first, stop=last)
                        if last:
                            nc.scalar.activation(
                                out=o_t[:, w * F:(w + 1) * F],
                                in_=ps_open.pop(w)[:],
                                func=mybir.ActivationFunctionType.Copy)
                # batched output write: one strided DMA for the full windows
                # + a tail DMA for the partial last window (saves ~96 HWDGE
                # fixed costs + semaphore round-trips vs per-window DMAs)
                nwf = npc // W                  # full windows
                rows_t = npc - nwf * W          # rows in partial last window
                nc.sync.dma_start(
                    out=out_d.ap()[:nwf * W, :].rearrange(
                        "(w r) f -> r w f", r=W),
                    in_=o_t[:, :nwf * F].rearrange(
                        "r (w f) -> r w f", f=F))
                if rows_t:
                    nc.sync.dma_start(
                        out=out_d.ap()[nwf * W:npc, :],
                        in_=o_t[:rows_t, nwf * F:(nwf + 1) * F])

            for _ in range(reps):
                body()
    nc.compile()
    return nc


def build_for_inputs(x, edge_weight, edge_index, reps=1, seq_idx=False,
                     **knobs):
    """Exposed for test harnesses: returns (nc, in_maps, meta)."""
    in_maps, meta = _host_prep(x, edge_weight, edge_index, seq_idx=seq_idx)
    nc = _build_program(meta, reps=reps, **knobs)
    return nc, in_maps, meta


def kernel(x, edge_weight, edge_index):
    x = np.asarray(x)
    nc, in_maps, _meta = build_for_inputs(x, edge_weight, edge_index)
    res = run_bass_kernel_spmd(nc, in_maps, core_ids=list(range(N_CORES)))
    out = np.concatenate(
        [res.results[c]["out"] for c in range(N_CORES)], axis=0)
    return out.astype(np.float32)
